# revision 25
# baseline (speedup 1.0000x reference)
"""EnergyAttention kernel for Trainium2 (Bass/Tile), 8-core data parallel.

Reference semantics:
    energy = einsum('bcd,d->bc', inputs, W[0]) + b[0]
    attn   = softmax(energy, axis=1)
    idx    = top_k(attn, 16).indices          # [B, K] descending
    out    = take_along_axis(inputs, idx)     # [B, K, D]

Key simplification: softmax is strictly monotonic per row and the bias is a
per-row constant shift, so top_k(attn) == top_k(energy) == top_k(dot(x, W)).
The output is just gathered input rows; no softmax/bias needed on device.

Per-core plan (B/8 = 32 batch rows, N = 32*512 = 16384 clip rows):
  1. Stream the rows through SBUF in natural-order 128-row tiles (contiguous
     DRAM -> full DMA bandwidth, ~357 GB/s measured); one fused DVE
     affine_mul_reduce per tile computes 128 dot products with W into
     column t of E1[128, 128].
  2. E1[p, 4b+chi] = energy[b, 128*chi + p]. Four TensorE transposes of the
     stride-4 column slices land E2[b, c] directly in PSUM (b on partitions,
     all 512 c in the free dim); ScalarE copies PSUM -> SBUF.
  3. Top-16 per row = two rounds of DVE max/max_index (top-8 each) with a
     match_replace(-inf) between rounds.
  4. One dma_gather of all 512 selected rows (4 KiB each) using indices
     idxs[k, b] = 512*b + idx[b, k] (int16), PE-transposed and PE-replicated
     8x down the 128 partitions (one copy per Q7 core); one 2 MiB store.
"""

import numpy as np

import concourse.bacc as bacc
import concourse.bass as bass
import concourse.mybir as mybir
import concourse.tile as tile
from concourse.bass_utils import run_bass_kernel_spmd
from concourse.masks import make_identity

B, C, D, K = 256, 512, 1024, 16
N_CORES = 8
BPC = B // N_CORES          # batch rows per core
N = BPC * C                 # clip rows per core
NT = N // 128               # 128 row-tiles per core

F32 = mybir.dt.float32
U32 = mybir.dt.uint32
I16 = mybir.dt.int16

NEG_INF = -3.0e38


def build_module() -> bass.Bass:
    nc = bacc.Bacc(None)
    x = nc.declare_dram_parameter("x", [N, D], F32, isOutput=False)
    w = nc.declare_dram_parameter("w", [128, D], F32, isOutput=False)
    y = nc.declare_dram_parameter("y", [BPC, K, D], F32, isOutput=True)
    y_flat = y[:].rearrange("b k d -> (b k) d")

    with tile.TileContext(nc) as tc:
        with (
            tc.tile_pool(name="xin", bufs=16) as xin_pool,
            tc.tile_pool(name="scr", bufs=2) as scr_pool,
            tc.tile_pool(name="small", bufs=1) as small_pool,
            tc.tile_pool(name="ps", bufs=1, space="PSUM") as ps_pool,
        ):
            w_sb = small_pool.tile([128, D], F32, tag="w")
            nc.sync.dma_start(out=w_sb[:], in_=w[:])
            ident = small_pool.tile([128, 128], F32, tag="ident")
            make_identity(nc, ident[:])
            # tiled identity: it16[p, m] = 1 iff m % 16 == p  (p < 16);
            # replicates a [16, S] block to all 128 partitions via matmul
            it16 = small_pool.tile([16, 128], F32, tag="it16")
            nc.gpsimd.memset(it16[:], 0.0)
            for c8 in range(8):
                nc.gpsimd.affine_select(
                    out=it16[:, 16 * c8 : 16 * (c8 + 1)],
                    in_=it16[:, 16 * c8 : 16 * (c8 + 1)],
                    compare_op=mybir.AluOpType.not_equal,
                    fill=1.0,
                    base=0,
                    pattern=[[-1, 16]],
                    channel_multiplier=1,
                )
            # nbase[b] = 512*b as f32 (exact below 2^24)
            nbase_u = small_pool.tile([BPC, 1], U32, tag="nbase_u")
            nc.gpsimd.iota(
                out=nbase_u[:], pattern=[[0, 1]], base=0, channel_multiplier=C
            )
            nbase_f = small_pool.tile([BPC, 1], F32, tag="nbase_f")
            nc.vector.tensor_copy(out=nbase_f[:], in_=nbase_u[:])

            # E1[p, t] = energy of row 128*t + p
            e1 = small_pool.tile([128, NT], F32, tag="e1")
            e1r = e1[:].rearrange("p (b chi) -> p chi b", chi=4)

            for t in range(NT):
                xt = xin_pool.tile([128, D], F32, tag="xt")
                nc.sync.dma_start(out=xt[:], in_=x[128 * t : 128 * (t + 1), :])
                # fused dot(x, W): out = (x*1 + 0)*W, accum = sum over free
                sc = scr_pool.tile([128, D], F32, tag="sc")
                nc.vector.affine_mul_reduce(
                    out=sc[:],
                    accum_out=e1[:, t : t + 1],
                    in0=xt[:],
                    in1=w_sb[:],
                    scale=1.0,
                    bias=0.0,
                )

            # e1[p, 4b + chi] = energy[b, 128*chi + p]. Four TensorE transposes
            # of the stride-4 column slices land E2[b, c] directly in PSUM.
            ps2 = ps_pool.tile([BPC, C], F32, tag="ps2")
            for chi in range(4):
                nc.tensor.transpose(
                    out=ps2[:, 128 * chi : 128 * (chi + 1)],
                    in_=e1r[:, chi, :],
                    identity=ident[:],
                )
            e2 = small_pool.tile([BPC, C], F32, tag="e2")
            nc.scalar.copy(out=e2[:], in_=ps2[:])

            # top-16 indices per batch row: two top-8 rounds
            m1 = small_pool.tile([BPC, 8], F32, tag="m1")
            m2 = small_pool.tile([BPC, 8], F32, tag="m2")
            idx = small_pool.tile([BPC, K], U32, tag="idx")
            e2b = small_pool.tile([BPC, C], F32, tag="e2b")
            nc.vector.max(out=m1[:], in_=e2[:])
            nc.vector.max_index(out=idx[:, 0:8], in_max=m1[:], in_values=e2[:])
            nc.vector.match_replace(
                out=e2b[:], in_to_replace=m1[:], in_values=e2[:], imm_value=NEG_INF
            )
            nc.vector.max(out=m2[:], in_=e2b[:])
            nc.vector.max_index(out=idx[:, 8:16], in_max=m2[:], in_values=e2b[:])

            # noff[b, k] = 512*b + idx[b, k] in f32 (exact below 2^24)
            idx_f = small_pool.tile([BPC, K], F32, tag="idx_f")
            nc.vector.tensor_copy(out=idx_f[:], in_=idx[:])
            noff_f = small_pool.tile([BPC, K], F32, tag="noff_f")
            nc.vector.tensor_scalar(
                out=noff_f[:],
                in0=idx_f[:],
                scalar1=nbase_f[:, 0:1],
                scalar2=None,
                op0=mybir.AluOpType.add,
            )

            # dma_gather wants idxs[k, b] (int16) tiled 8x down 128 partitions
            # (one copy per Q7 core): PE-transpose then PE-replicate.
            ps_t = ps_pool.tile([K, BPC], F32, tag="ps_t")
            nc.tensor.transpose(
                out=ps_t[:], in_=noff_f[:], identity=ident[0:BPC, 0:BPC]
            )
            nofft = small_pool.tile([K, BPC], F32, tag="nofft")
            nc.scalar.copy(out=nofft[:], in_=ps_t[:])
            ps_r = ps_pool.tile([128, BPC], F32, tag="ps_r")
            nc.tensor.matmul(out=ps_r[:], lhsT=it16[:], rhs=nofft[:])
            idxs16 = small_pool.tile([128, BPC], I16, tag="idxs16")
            nc.vector.tensor_copy(out=idxs16[:], in_=ps_r[:])

            # one fused gather of all 512 selected rows; item i = 16*b + k
            # lands at go[i % 128, i // 128, :]; one 2 MiB store
            NI = BPC * K
            go = small_pool.tile([128, NI // 128, D], F32, tag="go")
            nc.gpsimd.dma_gather(
                out_ap=go[:],
                in_ap=x[:],
                idxs_ap=idxs16[:],
                num_idxs=NI,
                num_idxs_reg=NI,
                elem_size=D,
            )
            y_r = y_flat.rearrange("(j p) d -> p j d", p=128)
            nc.sync.dma_start(out=y_r, in_=go[:])

    nc.finalize()
    return nc


_NC_CACHE: list = []


def _get_nc() -> bass.Bass:
    if not _NC_CACHE:
        _NC_CACHE.append(build_module())
    return _NC_CACHE[0]


def make_in_maps(inputs: np.ndarray, W: np.ndarray) -> list[dict]:
    w_rep = np.ascontiguousarray(
        np.broadcast_to(W.reshape(1, D).astype(np.float32, copy=False), (128, D))
    )
    return [
        {
            "x": np.ascontiguousarray(
                inputs[c * BPC : (c + 1) * BPC].reshape(N, D).astype(np.float32, copy=False)
            ),
            "w": w_rep,
        }
        for c in range(N_CORES)
    ]


def kernel(**inputs) -> np.ndarray:
    x_full = np.asarray(inputs["inputs"], dtype=np.float32)
    W = np.asarray(inputs["W"], dtype=np.float32)
    assert x_full.shape == (B, C, D), x_full.shape
    assert int(np.asarray(inputs["topk"])) == K
    nc = _get_nc()
    res = run_bass_kernel_spmd(nc, make_in_maps(x_full, W), core_ids=list(range(N_CORES)))
    return np.concatenate([res.results[c]["y"] for c in range(N_CORES)], axis=0).reshape(
        B, K, D
    )


# revision 26
# speedup vs baseline: 1.0172x; 1.0172x over previous
"""EnergyAttention kernel for Trainium2 (Bass/Tile), 8-core data parallel.

Reference semantics:
    energy = einsum('bcd,d->bc', inputs, W[0]) + b[0]
    attn   = softmax(energy, axis=1)
    idx    = top_k(attn, 16).indices          # [B, K] descending
    out    = take_along_axis(inputs, idx)     # [B, K, D]

Key simplification: softmax is strictly monotonic per row and the bias is a
per-row constant shift, so top_k(attn) == top_k(energy) == top_k(dot(x, W)).
The output is just gathered input rows; no softmax/bias needed on device.

Per-core plan (B/8 = 32 batch rows, N = 32*512 = 16384 clip rows):
  1. Stream the rows through SBUF in natural-order 128-row tiles (contiguous
     DRAM -> full DMA bandwidth, ~357 GB/s measured); one fused DVE
     affine_mul_reduce per tile computes 128 dot products with W into
     column t of E1[128, 128].
  2. E1[p, 4b+chi] = energy[b, 128*chi + p]. Four TensorE transposes of the
     stride-4 column slices land E2[b, c] directly in PSUM (b on partitions,
     all 512 c in the free dim); ScalarE copies PSUM -> SBUF.
  3. Top-16 per row = two rounds of DVE max/max_index (top-8 each) with a
     match_replace(-inf) between rounds.
  4. One dma_gather of all 512 selected rows (4 KiB each) using indices
     idxs[k, b] = 512*b + idx[b, k] (int16), PE-transposed and PE-replicated
     8x down the 128 partitions (one copy per Q7 core); one 2 MiB store.
"""

import numpy as np

import concourse.bacc as bacc
import concourse.bass as bass
import concourse.mybir as mybir
import concourse.tile as tile
from concourse.bass_utils import run_bass_kernel_spmd
from concourse.masks import make_identity

B, C, D, K = 256, 512, 1024, 16
N_CORES = 8
BPC = B // N_CORES          # batch rows per core
N = BPC * C                 # clip rows per core
NT = N // 128               # 128 row-tiles per core

F32 = mybir.dt.float32
U32 = mybir.dt.uint32
I16 = mybir.dt.int16

NEG_INF = -3.0e38


def build_module() -> bass.Bass:
    nc = bacc.Bacc(None)
    x = nc.declare_dram_parameter("x", [N, D], F32, isOutput=False)
    w = nc.declare_dram_parameter("w", [128, D], F32, isOutput=False)
    y = nc.declare_dram_parameter("y", [BPC, K, D], F32, isOutput=True)
    y_flat = y[:].rearrange("b k d -> (b k) d")

    with tile.TileContext(nc) as tc:
        with (
            tc.tile_pool(name="xin", bufs=16) as xin_pool,
            tc.tile_pool(name="scr", bufs=2) as scr_pool,
            tc.tile_pool(name="small", bufs=1) as small_pool,
            tc.tile_pool(name="ps", bufs=1, space="PSUM") as ps_pool,
        ):
            w_sb = small_pool.tile([128, D], F32, tag="w")
            nc.sync.dma_start(out=w_sb[:], in_=w[:])
            ident = small_pool.tile([128, 128], F32, tag="ident")
            make_identity(nc, ident[:])
            # tiled identity: it16[p, m] = 1 iff m % 16 == p  (p < 16);
            # replicates a [16, S] block to all 128 partitions via matmul
            it16 = small_pool.tile([16, 128], F32, tag="it16")
            nc.gpsimd.memset(it16[:], 0.0)
            for c8 in range(8):
                nc.gpsimd.affine_select(
                    out=it16[:, 16 * c8 : 16 * (c8 + 1)],
                    in_=it16[:, 16 * c8 : 16 * (c8 + 1)],
                    compare_op=mybir.AluOpType.not_equal,
                    fill=1.0,
                    base=0,
                    pattern=[[-1, 16]],
                    channel_multiplier=1,
                )
            # nbase[b] = 512*b as f32 (exact below 2^24)
            nbase_u = small_pool.tile([BPC, 1], U32, tag="nbase_u")
            nc.gpsimd.iota(
                out=nbase_u[:], pattern=[[0, 1]], base=0, channel_multiplier=C
            )
            nbase_f = small_pool.tile([BPC, 1], F32, tag="nbase_f")
            nc.vector.tensor_copy(out=nbase_f[:], in_=nbase_u[:])

            # E1[p, t] = energy of row 128*t + p
            e1 = small_pool.tile([128, NT], F32, tag="e1")
            e1r = e1[:].rearrange("p (b chi) -> p chi b", chi=4)

            for t in range(NT):
                xt = xin_pool.tile([128, D], F32, tag="xt")
                nc.sync.dma_start(out=xt[:], in_=x[128 * t : 128 * (t + 1), :])
                # fused dot(x, W): out = (x*1 + 0)*W, accum = sum over free
                sc = scr_pool.tile([128, D], F32, tag="sc")
                nc.vector.affine_mul_reduce(
                    out=sc[:],
                    accum_out=e1[:, t : t + 1],
                    in0=xt[:],
                    in1=w_sb[:],
                    scale=1.0,
                    bias=0.0,
                )

            # e1[p, 4b + chi] = energy[b, 128*chi + p]. Four TensorE transposes
            # of the stride-4 column slices land E2[b, c] directly in PSUM.
            ps2 = ps_pool.tile([BPC, C], F32, tag="ps2")
            for chi in range(4):
                nc.tensor.transpose(
                    out=ps2[:, 128 * chi : 128 * (chi + 1)],
                    in_=e1r[:, chi, :],
                    identity=ident[:],
                )
            e2 = small_pool.tile([BPC, C], F32, tag="e2")
            nc.scalar.copy(out=e2[:], in_=ps2[:])

            # top-16 indices per batch row: two top-8 rounds
            m1 = small_pool.tile([BPC, 8], F32, tag="m1")
            m2 = small_pool.tile([BPC, 8], F32, tag="m2")
            idx = small_pool.tile([BPC, K], U32, tag="idx")
            e2b = small_pool.tile([BPC, C], F32, tag="e2b")
            nc.vector.max(out=m1[:], in_=e2[:])
            nc.vector.max_index(out=idx[:, 0:8], in_max=m1[:], in_values=e2[:])
            nc.vector.match_replace(
                out=e2b[:], in_to_replace=m1[:], in_values=e2[:], imm_value=NEG_INF
            )
            nc.vector.max(out=m2[:], in_=e2b[:])
            nc.vector.max_index(out=idx[:, 8:16], in_max=m2[:], in_values=e2b[:])

            # noff[b, k] = 512*b + idx[b, k] in f32 (exact below 2^24)
            idx_f = small_pool.tile([BPC, K], F32, tag="idx_f")
            nc.vector.tensor_copy(out=idx_f[:], in_=idx[:])
            noff_f = small_pool.tile([BPC, K], F32, tag="noff_f")
            nc.vector.tensor_scalar(
                out=noff_f[:],
                in0=idx_f[:],
                scalar1=nbase_f[:, 0:1],
                scalar2=None,
                op0=mybir.AluOpType.add,
            )

            # dma_gather wants idxs[k, b] (int16) tiled 8x down 128 partitions
            # (one copy per Q7 core): PE-transpose then PE-replicate.
            ps_t = ps_pool.tile([K, BPC], F32, tag="ps_t")
            nc.tensor.transpose(
                out=ps_t[:], in_=noff_f[:], identity=ident[0:BPC, 0:BPC]
            )
            nofft = small_pool.tile([K, BPC], F32, tag="nofft")
            nc.scalar.copy(out=nofft[:], in_=ps_t[:])
            ps_r = ps_pool.tile([128, BPC], F32, tag="ps_r")
            nc.tensor.matmul(out=ps_r[:], lhsT=it16[:], rhs=nofft[:])
            idxs16 = small_pool.tile([128, BPC], I16, tag="idxs16")
            nc.vector.tensor_copy(out=idxs16[:], in_=ps_r[:])

            # fused gather of the 512 selected rows in two halves so the
            # first store overlaps the second gather; item i = 16*b + k
            # lands at go[i % 128, i // 128, :]
            NI = BPC * K
            y_r = y_flat.rearrange("(h j p) d -> h p j d", h=2, p=128)
            for h in range(2):
                go = small_pool.tile([128, NI // 256, D], F32, tag=f"go{h}")
                nc.gpsimd.dma_gather(
                    out_ap=go[:],
                    in_ap=x[:],
                    idxs_ap=idxs16[:, 16 * h : 16 * (h + 1)],
                    num_idxs=NI // 2,
                    num_idxs_reg=NI // 2,
                    elem_size=D,
                )
                nc.sync.dma_start(out=y_r[h], in_=go[:])

    nc.finalize()
    return nc


_NC_CACHE: list = []


def _get_nc() -> bass.Bass:
    if not _NC_CACHE:
        _NC_CACHE.append(build_module())
    return _NC_CACHE[0]


def make_in_maps(inputs: np.ndarray, W: np.ndarray) -> list[dict]:
    w_rep = np.ascontiguousarray(
        np.broadcast_to(W.reshape(1, D).astype(np.float32, copy=False), (128, D))
    )
    return [
        {
            "x": np.ascontiguousarray(
                inputs[c * BPC : (c + 1) * BPC].reshape(N, D).astype(np.float32, copy=False)
            ),
            "w": w_rep,
        }
        for c in range(N_CORES)
    ]


def kernel(**inputs) -> np.ndarray:
    x_full = np.asarray(inputs["inputs"], dtype=np.float32)
    W = np.asarray(inputs["W"], dtype=np.float32)
    assert x_full.shape == (B, C, D), x_full.shape
    assert int(np.asarray(inputs["topk"])) == K
    nc = _get_nc()
    res = run_bass_kernel_spmd(nc, make_in_maps(x_full, W), core_ids=list(range(N_CORES)))
    return np.concatenate([res.results[c]["y"] for c in range(N_CORES)], axis=0).reshape(
        B, K, D
    )


# revision 29
# speedup vs baseline: 1.0211x; 1.0038x over previous
"""EnergyAttention kernel for Trainium2 (Bass/Tile), 8-core data parallel.

Reference semantics:
    energy = einsum('bcd,d->bc', inputs, W[0]) + b[0]
    attn   = softmax(energy, axis=1)
    idx    = top_k(attn, 16).indices          # [B, K] descending
    out    = take_along_axis(inputs, idx)     # [B, K, D]

Key simplification: softmax is strictly monotonic per row and the bias is a
per-row constant shift, so top_k(attn) == top_k(energy) == top_k(dot(x, W)).
The output is just gathered input rows; no softmax/bias needed on device.

Per-core plan (B/8 = 32 batch rows, N = 32*512 = 16384 clip rows):
  1. Stream the rows through SBUF in natural-order 128-row tiles (contiguous
     DRAM -> full DMA bandwidth, ~357 GB/s measured); one fused DVE
     affine_mul_reduce per tile computes 128 dot products with W into
     column t of E1[128, 128].
  2. E1[p, 4b+chi] = energy[b, 128*chi + p]. Four TensorE transposes of the
     stride-4 column slices land E2[b, c] directly in PSUM (b on partitions,
     all 512 c in the free dim); ScalarE copies PSUM -> SBUF.
  3. Top-16 per row = two rounds of DVE max/max_index (top-8 each) with a
     match_replace(-inf) between rounds.
  4. One dma_gather of all 512 selected rows (4 KiB each) using indices
     idxs[k, b] = 512*b + idx[b, k] (int16), PE-transposed and PE-replicated
     8x down the 128 partitions (one copy per Q7 core); one 2 MiB store.
"""

import numpy as np

import concourse.bacc as bacc
import concourse.bass as bass
import concourse.mybir as mybir
import concourse.tile as tile
from concourse.bass_utils import run_bass_kernel_spmd
from concourse.masks import make_identity

B, C, D, K = 256, 512, 1024, 16
N_CORES = 8
BPC = B // N_CORES          # batch rows per core
N = BPC * C                 # clip rows per core
NT = N // 128               # 128 row-tiles per core

F32 = mybir.dt.float32
U32 = mybir.dt.uint32
I16 = mybir.dt.int16

NEG_INF = -3.0e38


def build_module() -> bass.Bass:
    nc = bacc.Bacc(None)
    x = nc.declare_dram_parameter("x", [N, D], F32, isOutput=False)
    w = nc.declare_dram_parameter("w", [128, D], F32, isOutput=False)
    y = nc.declare_dram_parameter("y", [BPC, K, D], F32, isOutput=True)
    y_flat = y[:].rearrange("b k d -> (b k) d")

    with tile.TileContext(nc) as tc:
        with (
            tc.tile_pool(name="xin", bufs=16) as xin_pool,
            tc.tile_pool(name="scr", bufs=2) as scr_pool,
            tc.tile_pool(name="small", bufs=1) as small_pool,
            tc.tile_pool(name="ps", bufs=1, space="PSUM") as ps_pool,
        ):
            w_sb = small_pool.tile([128, D], F32, tag="w")
            nc.sync.dma_start(out=w_sb[:], in_=w[:])
            ident = small_pool.tile([128, 128], F32, tag="ident")
            make_identity(nc, ident[:])
            # tiled identity: it16[p, m] = 1 iff m % 16 == p  (p < 16);
            # replicates a [16, S] block to all 128 partitions via matmul
            it16 = small_pool.tile([16, 128], F32, tag="it16")
            nc.gpsimd.memset(it16[:], 0.0)
            for c8 in range(8):
                nc.gpsimd.affine_select(
                    out=it16[:, 16 * c8 : 16 * (c8 + 1)],
                    in_=it16[:, 16 * c8 : 16 * (c8 + 1)],
                    compare_op=mybir.AluOpType.not_equal,
                    fill=1.0,
                    base=0,
                    pattern=[[-1, 16]],
                    channel_multiplier=1,
                )
            # nbase[b] = 512*b as f32 (exact below 2^24)
            nbase_u = small_pool.tile([16, 1], U32, tag="nbase_u")
            nc.gpsimd.iota(
                out=nbase_u[:], pattern=[[0, 1]], base=0, channel_multiplier=C
            )
            nbase_f = small_pool.tile([16, 1], F32, tag="nbase_f")
            nc.vector.tensor_copy(out=nbase_f[:], in_=nbase_u[:])

            # E1[p, t] = energy of row 128*t + p
            e1 = small_pool.tile([128, NT], F32, tag="e1")
            e1r = e1[:].rearrange("p (b chi) -> p chi b", chi=4)

            ps_t = ps_pool.tile([K, BPC], F32, tag="ps_t")
            nofft = small_pool.tile([K, BPC], F32, tag="nofft")

            def topk_half(hb):
                # per-half tiles, all at partition 0 (transpose matmul
                # outputs must land at PSUM partition 0)
                bb = 16 * hb
                ps2 = ps_pool.tile([16, C], F32, tag=f"ps2_{hb}")
                e2 = small_pool.tile([16, C], F32, tag=f"e2_{hb}")
                e2b = small_pool.tile([16, C], F32, tag=f"e2b_{hb}")
                m1 = small_pool.tile([16, 8], F32, tag=f"m1_{hb}")
                m2 = small_pool.tile([16, 8], F32, tag=f"m2_{hb}")
                idx = small_pool.tile([16, K], U32, tag=f"idx_{hb}")
                idx_f = small_pool.tile([16, K], F32, tag=f"idx_f_{hb}")
                noff_f = small_pool.tile([16, K], F32, tag=f"noff_f_{hb}")
                for chi in range(4):
                    nc.tensor.transpose(
                        out=ps2[:, 128 * chi : 128 * (chi + 1)],
                        in_=e1r[:, chi, bb : bb + 16],
                        identity=ident[:],
                    )
                nc.scalar.copy(out=e2[:], in_=ps2[:])
                nc.vector.max(out=m1[:], in_=e2[:])
                nc.vector.max_index(out=idx[:, 0:8], in_max=m1[:], in_values=e2[:])
                nc.vector.match_replace(
                    out=e2b[:], in_to_replace=m1[:], in_values=e2[:],
                    imm_value=NEG_INF,
                )
                nc.vector.max(out=m2[:], in_=e2b[:])
                nc.vector.max_index(out=idx[:, 8:16], in_max=m2[:], in_values=e2b[:])
                # noff[b, k] = 512*b + idx[b, k] in f32 (exact below 2^24)
                nc.vector.tensor_copy(out=idx_f[:], in_=idx[:])
                noff_f_ = noff_f
                nc.vector.tensor_scalar(
                    out=noff_f_[:],
                    in0=idx_f[:],
                    scalar1=nbase_f[0:16, 0:1],
                    scalar2=float(C * 16 * hb),
                    op0=mybir.AluOpType.add,
                    op1=mybir.AluOpType.add,
                )
                # idxs[k, b] block: PE-transpose [16, K] -> [K, 16]
                nc.tensor.transpose(
                    out=ps_t[:, bb : bb + 16],
                    in_=noff_f_[:],
                    identity=ident[0:16, 0:16],
                )
                nc.scalar.copy(out=nofft[:, bb : bb + 16], in_=ps_t[:, bb : bb + 16])

            for t in range(NT):
                xt = xin_pool.tile([128, D], F32, tag="xt")
                nc.sync.dma_start(out=xt[:], in_=x[128 * t : 128 * (t + 1), :])
                # fused dot(x, W): out = (x*1 + 0)*W, accum = sum over free
                sc = scr_pool.tile([128, D], F32, tag="sc")
                nc.vector.affine_mul_reduce(
                    out=sc[:],
                    accum_out=e1[:, t : t + 1],
                    in0=xt[:],
                    in1=w_sb[:],
                    scale=1.0,
                    bias=0.0,
                )
                if t == NT // 2 - 1:
                    # first 16 batch rows are complete: their fold + topk +
                    # index prep overlaps the second half of the load stream
                    topk_half(0)
            topk_half(1)

            # replicate nofft 8x down the partitions (one copy per Q7 core)
            ps_r = ps_pool.tile([128, BPC], F32, tag="ps_r")
            nc.tensor.matmul(out=ps_r[:], lhsT=it16[:], rhs=nofft[:])
            idxs16 = small_pool.tile([128, BPC], I16, tag="idxs16")
            nc.vector.tensor_copy(out=idxs16[:], in_=ps_r[:])

            # fused gather of the 512 selected rows in two halves so the
            # first store overlaps the second gather; item i = 16*b + k
            # lands at go[i % 128, i // 128, :]
            NI = BPC * K
            y_r = y_flat.rearrange("(h j p) d -> h p j d", h=2, p=128)
            for h in range(2):
                go = small_pool.tile([128, NI // 256, D], F32, tag=f"go{h}")
                nc.gpsimd.dma_gather(
                    out_ap=go[:],
                    in_ap=x[:],
                    idxs_ap=idxs16[:, 16 * h : 16 * (h + 1)],
                    num_idxs=NI // 2,
                    num_idxs_reg=NI // 2,
                    elem_size=D,
                )
                nc.sync.dma_start(out=y_r[h], in_=go[:])

    nc.finalize()
    return nc


_NC_CACHE: list = []


def _get_nc() -> bass.Bass:
    if not _NC_CACHE:
        _NC_CACHE.append(build_module())
    return _NC_CACHE[0]


def make_in_maps(inputs: np.ndarray, W: np.ndarray) -> list[dict]:
    w_rep = np.ascontiguousarray(
        np.broadcast_to(W.reshape(1, D).astype(np.float32, copy=False), (128, D))
    )
    return [
        {
            "x": np.ascontiguousarray(
                inputs[c * BPC : (c + 1) * BPC].reshape(N, D).astype(np.float32, copy=False)
            ),
            "w": w_rep,
        }
        for c in range(N_CORES)
    ]


def kernel(**inputs) -> np.ndarray:
    x_full = np.asarray(inputs["inputs"], dtype=np.float32)
    W = np.asarray(inputs["W"], dtype=np.float32)
    assert x_full.shape == (B, C, D), x_full.shape
    assert int(np.asarray(inputs["topk"])) == K
    nc = _get_nc()
    res = run_bass_kernel_spmd(nc, make_in_maps(x_full, W), core_ids=list(range(N_CORES)))
    return np.concatenate([res.results[c]["y"] for c in range(N_CORES)], axis=0).reshape(
        B, K, D
    )
